# revision 43
# baseline (speedup 1.0000x reference)
"""Trainium2 Bass kernel for a 2-layer GCN (segment-sum aggregation).

out = softmax( A @ relu(A @ h @ W1 + b1) @ W2 + b2 ),  A = adjacency (+self loops)

Strategy (8 NeuronCores, node/data parallel):
  - Nodes sharded by range: core k owns nodes [k*12500, (k+1)*12500).
  - Edges routed (on host) to the core owning their dst node, grouped by
    (128-node dst window, src%4 phase). Per-(window,phase) slots are padded
    to the max count over the 8 cores so one SPMD program serves all cores.
  - Layer 1 on device: dma_gather of h[src] rows (256B) from an HBM table
    viewed as [N/4, 1024B] super-rows (dodges the signed-int16 index limit);
    per 128-edge chunk a one-hot [edge x node] matrix is built on the
    VectorEngine (is_equal vs iota) and the TensorEngine accumulates
    aggT = sum featsT @ onehot into PSUM per window (segment-sum).
    Self loops are a dense add.  Then x1T = relu(W1.T @ aggT + b1) and
    y = x1T.T @ W2 (transform BEFORE layer-2 aggregation: 128 -> 40 dims).
  - AllGather of per-core y slices -> full y table (same super-row layout).
    Tuned (KERNEL_FLAGS): cc40 moves only the C=40 real columns (8MB instead
    of 12.8MB) and expands into a host-pre-zeroed 64-col table on device;
    gathers run multi-packet (single_packet=False). Both HW-measured wins.
  - Layer 2: same gathers/one-hots vs the y table, orientation flipped to
    give node-major [128,64] windows; + self loop + b2; softmax on chip.
"""

import math
import numpy as np

D = 64          # input feature dim (one gather row = 256B)
HID = 128
C = 40
CORES = 8
WIN = 128       # dst window (nodes per one-hot matmul window)
NPHASE = 4      # src mod-4 phases (int16 gather index reach)
GROUP = 4       # windows per gather instruction group


# ----------------------------------------------------------------------------
# Host-side routing
# ----------------------------------------------------------------------------

def cc_chunk_bounds(nw, nloc, q):
    """Split windows into q chunks -> list of (a, b) local-node row ranges."""
    res = []
    for i in range(q):
        w0 = (nw * i) // q
        w1 = (nw * (i + 1)) // q
        res.append((w0 * WIN, min(w1 * WIN, nloc)))
    return res


def route_edges(src, dst, n_nodes, cores=CORES, cc_chunks=1, group=GROUP):
    """Group edges by (core, window, phase) into padded static slots.

    Returns dict with static slot table S [NW, NPHASE] (python ints, shared by
    all cores), and per-core int16 index streams / fp32 dst streams.
    """
    nloc = n_nodes // cores
    nw = math.ceil(nloc / WIN)
    src = src.astype(np.int64)
    dst = dst.astype(np.int64)
    core = dst // nloc
    dloc = dst % nloc
    w = dloc // WIN
    phi = src % NPHASE
    key = (w * NPHASE + phi).astype(np.int64)   # per-core key in [0, nw*4)

    counts = np.zeros((cores, nw * NPHASE), np.int64)
    for k in range(cores):
        counts[k] = np.bincount(key[core == k], minlength=nw * NPHASE)
    nmax = counts.max(axis=0)                   # [nw*4]
    S = ((nmax + WIN - 1) // WIN * WIN).astype(np.int64)  # slot sizes, %128
    # ensure every window has at least one chunk (avoids empty PSUM groups)
    for wi in range(nw):
        if S[wi * NPHASE:(wi + 1) * NPHASE].sum() == 0:
            S[wi * NPHASE] = WIN

    # stream order: for group g: for phi: for w in group: slot(w, phi)
    ngroups = math.ceil(nw / group)
    order = []                                  # flat slot order: (w, phi)
    for g in range(ngroups):
        ws = range(g * group, min((g + 1) * group, nw))
        for p in range(NPHASE):
            for wi in ws:
                order.append(wi * NPHASE + p)
    order = np.array(order, np.int64)
    offs = np.zeros(nw * NPHASE, np.int64)      # slot -> stream offset
    offs[order] = np.r_[0, np.cumsum(S[order])[:-1]]
    tot = int(S.sum())

    # y4 super-row index per node: rank-major (cc_chunks==1) or
    # chunk-major/rank/local (chunked AllGather writes y4 chunk by chunk)
    if cc_chunks > 1:
        bounds = cc_chunk_bounds(nw, nloc, cc_chunks)
        aqs = np.array([a for a, b in bounds], np.int64)
        rows = np.array([b - a for a, b in bounds], np.int64)

        def y4row(n):
            c, loc = n // nloc, n % nloc
            q = np.searchsorted(aqs, loc, side="right") - 1
            return 2 * aqs[q] + c * (rows[q] // 4) + (loc - aqs[q]) // 4
    else:
        bounds = None

        def y4row(n):
            return n >> 2

    idx_streams, idx2_streams, dst_streams = [], [], []
    for k in range(cores):
        sel = core == k
        kk = key[sel]
        sidx = np.argsort(kk, kind="stable")
        kk_s = kk[sidx]
        # occurrence rank within each key group
        occ = np.arange(len(kk_s)) - np.repeat(
            np.r_[0, np.cumsum(np.bincount(kk_s, minlength=nw * NPHASE))[:-1]][kk_s], 1)
        pos = offs[kk_s] + occ
        idx = np.zeros(tot, np.int16)           # pad: super-row 0 (valid)
        idx2 = np.zeros(tot, np.int16)
        dsl = np.full(tot, -1000.0, np.float32)  # pad: no one-hot match
        srt = src[sel][sidx]
        idx[pos] = (srt >> 2).astype(np.int16)
        idx2[pos] = y4row(srt).astype(np.int16)
        dsl[pos] = (dloc[sel][sidx] % WIN).astype(np.float32)
        idx_streams.append(idx)
        idx2_streams.append(idx2)
        dst_streams.append(dsl)
    return dict(S=S, offs=offs, tot=tot, nw=nw, nloc=nloc,
                ngroups=ngroups, idx=idx_streams, idx2=idx2_streams,
                dst=dst_streams, cc_chunks=cc_chunks, cc_bounds=bounds,
                group=group)


# ----------------------------------------------------------------------------
# Bass program
# ----------------------------------------------------------------------------

def build_program(n_nodes, rt, do_cc=True, l2_table_y=True, maxidx=1024,
                  scratch=16384, skip_compute=False, skip_gather=False,
                  elem512=False, gbufs=3, single_packet=True, cc_chunks=1,
                  cc40=False, sbufA=False, group=GROUP):
    import concourse.bass as bass
    import concourse.mybir as mybir
    import concourse.bacc as bacc
    from concourse import tile

    f32 = mybir.dt.float32
    bf16 = mybir.dt.float16
    i16 = mybir.dt.int16
    S, offs, tot, nw, nloc, ngroups = (rt["S"], rt["offs"], rt["tot"],
                                       rt["nw"], rt["nloc"], rt["ngroups"])
    nch = tot // WIN                       # total chunks
    nsup = n_nodes // NPHASE               # super-rows in gather tables
    nlocp = nw * WIN                       # padded local node count
    last_rows = nloc - (nw - 1) * WIN      # rows in the last window

    nc = bacc.Bacc(None, target_bir_lowering=False, debug=False,
                   num_swdge_queues=4, dynamic_dma_scratch_size=scratch)

    h4 = nc.declare_dram_parameter("h4", [nsup, NPHASE * D], bf16, False)
    NRANK = math.ceil(nsup / 128)          # sbufA table ranks (128 tok each)
    h4sbd = (nc.declare_dram_parameter(
        "h4sb", [128, NRANK * NPHASE * D], bf16, False) if sbufA else None)
    eyed = (nc.declare_dram_parameter("eye64", [2 * D, D], bf16, False)
            if sbufA else None)
    hTo = nc.declare_dram_parameter("hTo", [D, nlocp], f32, False)
    W1d = nc.declare_dram_parameter("W1", [D, HID], f32, False)
    b1d = nc.declare_dram_parameter("b1", [HID, 1], f32, False)
    W2d = nc.declare_dram_parameter("W2p", [HID, D], f32, False)
    b2d = nc.declare_dram_parameter("b2b", [WIN, D], f32, False)
    idxd = nc.declare_dram_parameter("idx", [128, tot // 16], i16, False)
    idx2d = (nc.declare_dram_parameter("idx2", [128, tot // 16], i16, False)
             if cc_chunks > 1 else None)
    dstd = nc.declare_dram_parameter("dstf", [WIN, nch], f32, False)
    iotad = nc.declare_dram_parameter("iota", [WIN, WIN], bf16, False)
    outd = nc.declare_dram_parameter("out", [nloc, C], f32, True)

    ccd = C if cc40 else D                 # cols moved by the AllGather
    cc_in = nc.dram_tensor("cc_in", [nloc, ccd], bf16)
    if cc40:
        # gather table arrives as a pre-zeroed input; expand_y40 fills :C
        y4 = nc.declare_dram_parameter(
            "y4z", [CORES * nloc // NPHASE, NPHASE * D], bf16, False)
        y40 = nc.dram_tensor("y40", [CORES * nloc, C], bf16,
                             addr_space="Shared")
    else:
        y4 = nc.dram_tensor("y4", [CORES * nloc // NPHASE, NPHASE * D], bf16,
                            addr_space="Shared")
        y40 = None

    def expand_y40(a, b):
        # pad 40-col allgathered rows into the 64-col/node gather table
        # (batched: walrus caps one AP dim at 65535 elements)
        step = 8192 * NPHASE            # nodes per DMA (8192 super-rows)
        for n0 in range(CORES * a, CORES * b, step):
            n1 = min(n0 + step, CORES * b)
            nc.sync.dma_start(
                y4[n0 // NPHASE:n1 // NPHASE, :]
                .rearrange("r (n f) -> r n f", n=NPHASE)[:, :, :C],
                y40[n0:n1, :].rearrange("(r n) f -> r n f", n=NPHASE))

    # slot geometry helpers -------------------------------------------------
    def group_windows(g):
        return range(g * group, min((g + 1) * group, nw))

    # per-(group) chunk layout inside the group's gather buffer
    gbase = {}     # (w, phi) -> (group, chunk col within group buffer)
    gchunks = []   # chunks per group
    for g in range(ngroups):
        col = 0
        for p in range(NPHASE):
            for wi in group_windows(g):
                gbase[(wi, p)] = (g, col)
                col += int(S[wi * NPHASE + p]) // WIN
        gchunks.append(col)

    ESIZE = NPHASE * D if elem512 else 2 * D   # gather element (bf16 elems)

    Relu = mybir.ActivationFunctionType.Relu
    Exp = mybir.ActivationFunctionType.Exp
    add_op = mybir.AluOpType.add
    eq_op = mybir.AluOpType.is_equal

    with tile.TileContext(nc) as tc:
        import contextlib
        with contextlib.ExitStack() as ctx:
            cpool = ctx.enter_context(tc.tile_pool(name="const", bufs=1))
            ypool = ctx.enter_context(tc.tile_pool(name="yown", bufs=1))

            fake_gt = None
            if skip_gather:
                fake_gt = cpool.tile([WIN, 4096], bf16)
                nc.scalar.memzero(fake_gt[:])

            idx_sb = cpool.tile([128, tot // 16], i16)
            if cc_chunks > 1:
                idx2_sb = cpool.tile([128, tot // 16], i16)
            else:
                idx2_sb = idx_sb
            dst_sb = cpool.tile([WIN, nch], f32)
            iota_sb = cpool.tile([WIN, WIN], bf16)
            if not sbufA:
                hTo_sb = cpool.tile([D, nlocp], f32)
            else:
                eye_sb = cpool.tile([2 * D, D], bf16)
            W1_sb = cpool.tile([D, HID], f32)
            b1_sb = cpool.tile([HID, 1], f32)
            W2_sb = cpool.tile([HID, D], f32)
            b2_sb = cpool.tile([WIN, D], f32)
            yown = ypool.tile([WIN, nw * D], f32)

            nc.sync.dma_start(idx_sb[:], idxd[:])
            if cc_chunks > 1:
                nc.sync.dma_start(idx2_sb[:], idx2d[:])
            nc.sync.dma_start(dst_sb[:], dstd[:])
            nc.sync.dma_start(iota_sb[:], iotad[:])
            if not sbufA:
                nc.sync.dma_start(hTo_sb[:], hTo[:])
            else:
                nc.sync.dma_start(eye_sb[:], eyed[:])
            nc.sync.dma_start(W1_sb[:], W1d[:])
            nc.sync.dma_start(b1_sb[:], b1d[:])
            nc.sync.dma_start(W2_sb[:], W2d[:])
            nc.sync.dma_start(b2_sb[:], b2d[:])

            MAXIDX = maxidx  # default 1024: 64 desc/engine x 16 engines/packet
            qctr = [0]      # round-robin SWDGE queue (4 Q7 core pairs)

            def issue_gathers(g, gt, table, isb=None, sbuf=False):
                if skip_gather:
                    return
                isb = idx_sb if isb is None else isb
                for p in range(NPHASE):
                    ws = list(group_windows(g))
                    n = int(sum(S[wi * NPHASE + p] for wi in ws))
                    if n == 0:
                        continue
                    o = int(offs[ws[0] * NPHASE + p])
                    _, col0 = gbase[(ws[0], p)]
                    j = p >> 1          # node-pair within the 4-node super-row
                    if not sbuf:
                        in_ap = (table[:] if elem512
                                 else table[:, j * 2 * D:(j + 1) * 2 * D])
                    for s0 in range(0, n, MAXIDX):
                        ni = min(MAXIDX, n - s0)
                        c0 = col0 + s0 // WIN
                        oo = o + s0
                        if sbuf:
                            # SBUF-source transpose gather: full 512B rows,
                            # feature-major [2, ni] block packed per call at
                            # element offset 2*pos0 (see chunk_fm for reads)
                            pos0 = c0 * WIN
                            nc.gpsimd.dma_gather(
                                out_ap=gt[:, 2 * pos0:2 * (pos0 + ni)]
                                .rearrange("p (c n) -> p c n", c=2),
                                in_ap=table[:],
                                idxs_ap=isb[:, oo // 16: (oo + ni) // 16],
                                num_idxs=ni,
                                num_idxs_reg=ni,
                                elem_size=NPHASE * D,
                                transpose=True,
                                sbuf_tokens_per_rank=128,
                                sbuf_free_dim_per_rank=NPHASE * D * 2,
                                single_packet=True,
                                queue_num=qctr[0] % 4,
                            )
                        else:
                            nc.gpsimd.dma_gather(
                                out_ap=gt[:, c0 * ESIZE:(c0 + ni // WIN)
                                          * ESIZE]
                                .rearrange("p (c f) -> p c f", f=ESIZE),
                                in_ap=in_ap,
                                idxs_ap=isb[:, oo // 16: (oo + ni) // 16],
                                num_idxs=ni,
                                num_idxs_reg=ni,
                                elem_size=ESIZE,
                                elem_step=NPHASE * D,
                                single_packet=single_packet,
                                queue_num=qctr[0] % 4,
                            )
                        qctr[0] += 1

            def window_chunks(wi, full_p=False):
                res = []
                for p in range(NPHASE):
                    g, col = gbase[(wi, p)]
                    for c in range(int(S[wi * NPHASE + p]) // WIN):
                        res.append((int(offs[wi * NPHASE + p]) // WIN + c,
                                    col + c,
                                    p if (elem512 or full_p) else p & 1))
                return res

            def chunk_fm(gt, g, lcol, p):
                # feature-major [64, WIN] view of one chunk inside the
                # call-packed sbuf transpose-gather buffer of group g
                ws0 = g * group
                run_col0 = gbase[(ws0, p)][1]
                run_n = int(sum(S[wi * NPHASE + p]
                                for wi in group_windows(g)))
                q = lcol * WIN
                rel = q - run_col0 * WIN
                s0 = (rel // MAXIDX) * MAXIDX
                pos0 = run_col0 * WIN + s0
                ni = min(MAXIDX, run_n - s0)
                off = q - pos0
                base = 2 * pos0 + (p >> 1) * ni + off
                return gt[(p & 1) * D:(p & 1) * D + D, base:base + WIN]

            def chunk_src(gt, i, lcol, ph):
                if skip_gather:
                    return fake_gt[:, (i % 63) * 64:(i % 63) * 64 + D]
                c0 = lcol * ESIZE + ph * D
                return gt[:, c0:c0 + D]

            # ---------------- stage A: layer 1 ----------------
            with contextlib.ExitStack() as sa:
                gpool = sa.enter_context(tc.tile_pool(name="gatherA", bufs=gbufs))
                ohpool = sa.enter_context(tc.tile_pool(name="ohA", bufs=16))
                aggpool = sa.enter_context(tc.tile_pool(name="aggT", bufs=4))
                xpool = sa.enter_context(tc.tile_pool(name="x1", bufs=4))
                psA = sa.enter_context(
                    tc.tile_pool(name="psA", bufs=2 if sbufA else 3,
                                 space="PSUM"))
                psB = sa.enter_context(
                    tc.tile_pool(name="psB", bufs=2, space="PSUM"))
                psC = sa.enter_context(
                    tc.tile_pool(name="psC", bufs=2, space="PSUM"))
                if sbufA:
                    h4pool = sa.enter_context(
                        tc.tile_pool(name="h4tab", bufs=1))
                    htpool = sa.enter_context(
                        tc.tile_pool(name="htw", bufs=4))
                    chpool = sa.enter_context(
                        tc.tile_pool(name="chunks", bufs=8))
                    psT = sa.enter_context(
                        tc.tile_pool(name="psT", bufs=2, space="PSUM"))
                    NRANK = math.ceil(nsup / 128)
                    h4t = h4pool.tile([128, NRANK * NPHASE * D], bf16)
                    nc.sync.dma_start(h4t[:], h4sbd[:])

                for g in range(ngroups):
                    if sbufA:
                        gt = gpool.tile([WIN, 2 * gchunks[g] * WIN], bf16,
                                        tag="gbuf")
                        issue_gathers(g, gt, h4t, sbuf=True)
                    else:
                        gt = gpool.tile([WIN, gchunks[g] * ESIZE], bf16,
                                        tag="gbuf")
                        issue_gathers(g, gt, h4)
                    for wi in group_windows(g):
                        if skip_compute:
                            lcol0 = gbase[(wi, 0)][1]
                            ybf = xpool.tile([WIN, D], bf16, tag="ybf")
                            nc.scalar.copy(
                                ybf[:],
                                gt[:, lcol0 * ESIZE:lcol0 * ESIZE + D])
                            nc.scalar.copy(yown[:, wi * D:(wi + 1) * D],
                                           b2_sb[:])
                            rows = last_rows if wi == nw - 1 else WIN
                            nc.sync.dma_start(
                                cc_in[wi * WIN: wi * WIN + rows, :],
                                ybf[:rows, :])
                            continue
                        chunks = window_chunks(wi, full_p=sbufA)
                        if sbufA:
                            htw = htpool.tile([D, WIN], f32)
                            nc.sync.dma_start(
                                htw[:], hTo[:, wi * WIN:(wi + 1) * WIN])
                        ps = psA.tile([D, WIN], f32)
                        for i, (gcol, lcol, half) in enumerate(chunks):
                            oh = ohpool.tile([WIN, WIN], bf16)
                            nc.vector.tensor_scalar(
                                oh[:], iota_sb[:], dst_sb[:, gcol:gcol + 1],
                                None, eq_op)
                            if sbufA:
                                hb = (half & 1) * D
                                tr = psT.tile([WIN, D], bf16)
                                nc.tensor.transpose(
                                    tr[:], chunk_fm(gt, g, lcol, half),
                                    eye_sb[hb:hb + D, :])
                                cht = chpool.tile([WIN, D], bf16)
                                nc.scalar.copy(cht[:], tr[:])
                                lhsT = cht[:]
                            else:
                                lhsT = chunk_src(gt, i, lcol, half)
                            nc.tensor.matmul(
                                ps[:], lhsT, oh[:],
                                start=(i == 0), stop=(i == len(chunks) - 1))
                        aggT = aggpool.tile([D, WIN], f32)
                        nc.vector.tensor_tensor(
                            aggT[:], ps[:],
                            htw[:] if sbufA
                            else hTo_sb[:, wi * WIN:(wi + 1) * WIN],
                            add_op)
                        ps2 = psB.tile([HID, WIN], f32)
                        nc.tensor.matmul(ps2[:], W1_sb[:], aggT[:])
                        x1 = xpool.tile([HID, WIN], f32)
                        nc.scalar.activation(x1[:], ps2[:], Relu,
                                             bias=b1_sb[:, 0:1])
                        ps3 = psC.tile([WIN, D], f32)
                        nc.tensor.matmul(ps3[:], x1[:], W2_sb[:])
                        nc.scalar.copy(yown[:, wi * D:(wi + 1) * D], ps3[:])
                        ybf = xpool.tile([WIN, D], bf16, tag="ybf")
                        nc.scalar.copy(ybf[:], ps3[:])
                        rows = last_rows if wi == nw - 1 else WIN
                        nc.sync.dma_start(
                            cc_in[wi * WIN: wi * WIN + rows, :],
                            ybf[:rows, :ccd])
                    if do_cc and cc_chunks > 1:
                        last_w = max(group_windows(g))
                        for q, (a, b) in enumerate(rt["cc_bounds"]):
                            bw = (b + WIN - 1) // WIN - 1   # last window of q
                            if bw == last_w:
                                nc.gpsimd.collective_compute(
                                    "AllGather", mybir.AluOpType.bypass,
                                    replica_groups=[list(range(CORES))],
                                    ins=[cc_in[a:b, :]],
                                    outs=[y40[CORES * a:CORES * b, :]
                                          if cc40 else y4[2 * a:2 * b, :]])
                                if cc40:
                                    expand_y40(a, b)

            # ---------------- all-gather of y ----------------
            if do_cc and cc_chunks == 1:
                nc.gpsimd.collective_compute(
                    "AllGather", mybir.AluOpType.bypass,
                    replica_groups=[list(range(CORES))],
                    ins=[cc_in.ap().opt()],
                    outs=[y40.ap().opt() if cc40 else y4.ap().opt()])
                if cc40:
                    expand_y40(0, nloc)

            # ---------------- stage C: layer 2 ----------------
            with contextlib.ExitStack() as sc:
                gpool = sc.enter_context(tc.tile_pool(name="gatherC", bufs=gbufs))
                ohpool = sc.enter_context(tc.tile_pool(name="ohC", bufs=16))
                spool = sc.enter_context(tc.tile_pool(name="smax", bufs=4))
                opool = sc.enter_context(tc.tile_pool(name="outp", bufs=3))
                psD = sc.enter_context(
                    tc.tile_pool(name="psD", bufs=4, space="PSUM"))

                for g in range(ngroups):
                    gt = gpool.tile([WIN, gchunks[g] * ESIZE], bf16, tag="gbufC")
                    issue_gathers(g, gt, y4 if l2_table_y else h4,
                                  isb=idx2_sb)
                    for wi in group_windows(g):
                        if skip_compute:
                            lcol0 = gbase[(wi, 0)][1]
                            o = opool.tile([WIN, C], f32)
                            nc.scalar.copy(
                                o[:], gt[:, lcol0 * ESIZE:lcol0 * ESIZE + C])
                            rows = last_rows if wi == nw - 1 else WIN
                            nc.sync.dma_start(
                                outd[wi * WIN: wi * WIN + rows, :],
                                o[:rows, :])
                            continue
                        chunks = window_chunks(wi)
                        ps = psD.tile([WIN, D], f32)
                        for i, (gcol, lcol, half) in enumerate(chunks):
                            oh = ohpool.tile([WIN, WIN], bf16)
                            nc.vector.tensor_scalar(
                                oh[:], iota_sb[:], dst_sb[:, gcol:gcol + 1],
                                None, eq_op)
                            nc.tensor.matmul(
                                ps[:], oh[:], chunk_src(gt, i, lcol, half),
                                start=(i == 0), stop=(i == len(chunks) - 1))
                        t1 = spool.tile([WIN, D], f32, tag="t1")
                        nc.vector.tensor_tensor(
                            t1[:], ps[:], yown[:, wi * D:(wi + 1) * D], add_op)
                        t2 = spool.tile([WIN, D], f32, tag="t2")
                        nc.vector.tensor_tensor(t2[:], t1[:], b2_sb[:], add_op)
                        mx = spool.tile([WIN, 1], f32, tag="mx")
                        nc.vector.tensor_reduce(
                            mx[:], t2[:, :C], mybir.AxisListType.X,
                            mybir.AluOpType.max, negate=True)
                        e = spool.tile([WIN, C], f32, tag="e")
                        nc.scalar.activation(e[:], t2[:, :C], Exp,
                                             bias=mx[:, 0:1])
                        sm = spool.tile([WIN, 1], f32, tag="sm")
                        nc.vector.tensor_reduce(
                            sm[:], e[:], mybir.AxisListType.X, add_op)
                        ri = spool.tile([WIN, 1], f32, tag="ri")
                        nc.vector.reciprocal(ri[:], sm[:])
                        o = opool.tile([WIN, C], f32)
                        nc.vector.tensor_scalar_mul(o[:], e[:], ri[:, 0:1])
                        rows = last_rows if wi == nw - 1 else WIN
                        nc.sync.dma_start(
                            outd[wi * WIN: wi * WIN + rows, :], o[:rows, :])

    nc.finalize()
    return nc


# ----------------------------------------------------------------------------
# Entry point
# ----------------------------------------------------------------------------

def _prepare_inputs(node_embeddings, adjacency_lists, W1, b1, W2, b2, rt,
                    cc40=False, sbufA=False):
    n, d = node_embeddings.shape
    nloc, nw = rt["nloc"], rt["nw"]
    nlocp = nw * WIN
    bf = np.float16
    h = np.ascontiguousarray(node_embeddings, np.float32)
    h4 = h.astype(bf).reshape(n // NPHASE, NPHASE * d)
    W2p = np.zeros((HID, D), np.float32)
    W2p[:, :C] = W2
    b2b = np.tile(np.pad(b2.astype(np.float32), (0, D - C)), (WIN, 1))
    iota = np.tile(np.arange(WIN, dtype=np.float32), (WIN, 1))
    in_maps = []
    for k in range(CORES):
        hTo = np.zeros((d, nlocp), np.float32)
        hTo[:, :nloc] = h[k * nloc:(k + 1) * nloc].T
        in_maps.append({
            "h4": h4,
            "hTo": hTo,
            "W1": np.ascontiguousarray(W1, np.float32),
            "b1": np.ascontiguousarray(b1, np.float32).reshape(HID, 1),
            "W2p": W2p,
            "b2b": b2b,
            "idx": np.tile(rt["idx"][k].reshape(-1, 16).T, (8, 1)).copy(),
            **({"idx2": np.tile(rt["idx2"][k].reshape(-1, 16).T,
                                (8, 1)).copy()}
               if rt["cc_chunks"] > 1 else {}),
            "dstf": np.ascontiguousarray(
                rt["dst"][k].reshape(-1, WIN).T),
            "iota": iota.astype(bf),
            **(_sbufa_inputs(h4) if sbufA else {}),
            **({"y4z": np.zeros((CORES * nloc // NPHASE, NPHASE * D), bf)}
               if cc40 else {}),
            "out": np.zeros((nloc, C), np.float32),
        })
    return in_maps


def _sbufa_inputs(h4):
    # token t (super-row) -> partition t%128, rank t//128 (sbuf-gather layout)
    nsup, row = h4.shape
    nrank = math.ceil(nsup / 128)
    pad = np.zeros((nrank * 128 - nsup, row), h4.dtype)
    h4sb = (np.concatenate([h4, pad]).reshape(nrank, 128, row)
            .transpose(1, 0, 2).reshape(128, nrank * row))
    return {"h4sb": np.ascontiguousarray(h4sb),
            "eye64": np.tile(np.eye(D, dtype=h4.dtype), (2, 1))}


_CACHE = {}


def _get_program(n_nodes, rt_sig, rt):
    key = (n_nodes, rt_sig)
    if key not in _CACHE:
        _CACHE[key] = build_program(n_nodes, rt)
    return _CACHE[key]


def build_all(node_embeddings, adjacency_lists, W1, b1, W2, b2,
              cache=True, **build_flags):
    """Route edges, build (cached) program, prepare per-core inputs."""
    n = node_embeddings.shape[0]
    src = np.asarray(adjacency_lists)[:, 0]
    dst = np.asarray(adjacency_lists)[:, 1]
    rt = route_edges(src, dst, n,
                     cc_chunks=build_flags.get("cc_chunks", 1),
                     group=build_flags.get("group", GROUP))
    if cache:
        rt_sig = (rt["tot"], tuple(rt["S"].tolist()),
                  tuple(sorted(build_flags.items())))
        key = (n, rt_sig)
        if key not in _CACHE:
            _CACHE[key] = build_program(n, rt, **build_flags)
        nc = _CACHE[key]
    else:
        nc = build_program(n, rt, **build_flags)
    in_maps = _prepare_inputs(node_embeddings, adjacency_lists,
                              W1, b1, W2, b2, rt,
                              cc40=build_flags.get("cc40", False),
                              sbufA=build_flags.get("sbufA", False))
    return nc, in_maps, rt


# tuned build configuration used by kernel()
KERNEL_FLAGS = {"cc40": True, "single_packet": False}


def kernel(node_embeddings, adjacency_lists, W1, b1, W2, b2, trace=False):
    import sys
    if "/opt/trn_rl_repo" not in sys.path:
        sys.path.insert(0, "/opt/trn_rl_repo")
    from concourse import bass_utils

    nc, in_maps, rt = build_all(node_embeddings, adjacency_lists,
                                W1, b1, W2, b2, **KERNEL_FLAGS)
    res = bass_utils.run_bass_kernel_spmd(
        nc, in_maps, core_ids=list(range(CORES)), trace=trace)
    out = np.concatenate([res.results[k]["out"] for k in range(CORES)], axis=0)
    kernel.last_result = res
    kernel.last_nc = nc
    kernel.last_in_maps = in_maps
    return out

